# revision 58
# baseline (speedup 1.0000x reference)
"""Trainium2 Bass kernel for ExactVisionAttention (block-diagonal attention).

Full inputs in, full outputs out. Sharding: segment-parallel over the 8
equal-length segments (attention is block-diagonal across segments), one
segment per NeuronCore. No collectives needed.

v2 dataflow per core (segment of 1024 tokens, HID=1280, 16 heads, D=80):
  Host prep (free): hidden pre-transposed to hidT and split into fp8-e4m3
  hi/lo pairs (power-of-2 prescaled); wqkv likewise; cos/sin/wproj in bf16.
  A: QKV via fp8 DoubleRow matmuls, 3 "clean" terms per 256-deep
     contraction pair: hi*hi + lo*hi + hi*lo  (~bf16 accuracy, 0.75x the
     row-cycles of bf16). PSUM evict on ACT with the descale folded in;
     RoPE on DVE in bf16 (4x mode); V eviction straight to bf16 (+ones col).
  B: per head on PE: transpose q,k to [80,1024] (bf16, 1cyc/row); S^T =
     kT.T@qT chunks; exp on ACT (scale folded, no max-subtraction);
     oT = [V|1].T@P^T accumulated; row 80 = softmax sums. Normalize via
     gpsimd partition-broadcast + DVE reciprocal/mul (bf16); pack the
     normalized aoT head rows into 128-partition e-major SBUF tiles with
     SBUF->SBUF DMA (no DRAM scratch round trip).
  C: output projection, also fp8 DoubleRow 3-term: the normalized head
     outputs are split hi/lo on DVE in phase B and packed into DoubleRow
     pair layout; wproj hi/lo comes prescaled from the host (prefetched
     during B). A leading 2-mt pair-outer group covers the last head's
     norm latency; descale folds into the f32 eviction before DMA out.

qkv_bias / proj_bias are zeros by construction (spec fill=zeros) and are
not applied. cu_seqlens is fixed equal segmentation and only validated.
"""

import os
import sys

for _p in ("/opt/trn_rl_repo", "/root/.axon_site", "/root/.axon_site/_ro/trn_rl_repo",
           "/root/.axon_site/_ro/pypackages"):
    if os.path.isdir(_p) and _p not in sys.path:
        sys.path.append(_p)

import numpy as np

S = 8192
HID = 1280
H = 16
D = 80
NSEG = 8
SEG = S // NSEG          # 1024 tokens per segment/core
MT = SEG // 128          # 8 token tiles per core
KC = HID // 128          # 10 proj contraction chunks
NP = 5                   # qkv contraction pairs (256 deep each)
CW = 480                 # qkv output column chunk (6 heads)
NCH = 3 * HID // CW      # 8 column chunks
SCALE = float(D) ** -0.5

_CACHE = {}


def _chunk_segments(c):
    """Column chunk c -> [(kind, head0, nheads, col_off_in_chunk)]."""
    segs = []
    col = c * CW
    end = col + CW
    while col < end:
        kind = col // HID
        base = kind * HID
        h0 = (col - base) // D
        ncols = min(end, base + HID) - col
        nh = ncols // D
        segs.append((kind, h0, nh, col - c * CW))
        col += nh * D
    return segs


SA = 64.0  # fixed pow2 prescale for the normalized attention output


def build_module(num_devices=8, repeat=1, inv_s=1.0 / (32.0 * 2048.0),
                 inv_p=1.0 / (64.0 * 2048.0)):
    import concourse.tile as tile
    from concourse import bacc, mybir

    f32 = mybir.dt.float32
    bf16 = mybir.dt.bfloat16
    f8 = mybir.dt.float8e4
    Exp = mybir.ActivationFunctionType.Exp
    DR = mybir.MatmulPerfMode.DoubleRow

    nc = bacc.Bacc("TRN2", target_bir_lowering=False, debug=False,
                   num_devices=num_devices)

    sta_in = nc.dram_tensor("sta8", [128, NP, 2, 2, SEG], f8,
                            kind="ExternalInput").ap()
    mov_in = nc.dram_tensor("mov8", [128, NCH, NP, 2, 2, CW], f8,
                            kind="ExternalInput").ap()
    cos_in = nc.dram_tensor("cosb", [128, MT, 40], bf16,
                            kind="ExternalInput").ap()
    sin_in = nc.dram_tensor("sinb", [128, MT, 40], bf16,
                            kind="ExternalInput").ap()
    wpj_in = nc.dram_tensor("wpj8", [128, NP, 2, 2, HID], f8,
                            kind="ExternalInput").ap()
    ident_in = nc.dram_tensor("identb", [128, 128], bf16,
                              kind="ExternalInput").ap()
    out_dram = nc.dram_tensor("out", [SEG, HID], f32, kind="ExternalOutput").ap()
    tag_dram = None
    if repeat > 1:
        tag_dram = nc.dram_tensor("rtag", [1, repeat], f32,
                                  kind="ExternalOutput").ap()

    with tile.TileContext(nc) as tc:
      from contextlib import ExitStack
      for _rep in range(repeat):
        with ExitStack() as ctx:
            constp = ctx.enter_context(tc.tile_pool(name="const", bufs=1))
            projp = ctx.enter_context(tc.tile_pool(name="projp", bufs=1))
            qkv_ctx = ExitStack()
            qkvsb = qkv_ctx.enter_context(tc.tile_pool(name="qkvsb", bufs=1))

            from concourse import library_config
            nc.gpsimd.load_library(library_config.proxy)

            q_sb = [qkvsb.tile([128, H, D], bf16, tag=f"q{mt}", name=f"q{mt}")
                    for mt in range(MT)]
            k_sb = [qkvsb.tile([128, H, D], bf16, tag=f"k{mt}", name=f"k{mt}")
                    for mt in range(MT)]
            v_sb = [qkvsb.tile([128, H, D + 1], bf16, tag=f"v{mt}", name=f"v{mt}")
                    for mt in range(MT)]

            if tag_dram is not None:
                nc.sync.dma_start(tag_dram[:, _rep:_rep + 1],
                                  cos_in[0:1, 0:1, 0:1].rearrange("a b c -> a (b c)"))

            # ---------------- Phase A: QKV (fp8 DoubleRow) + RoPE ----------
            with ExitStack() as actx:
                stap = actx.enter_context(tc.tile_pool(name="stap", bufs=1))
                movp = actx.enter_context(tc.tile_pool(name="movp", bufs=3))
                qsp = actx.enter_context(tc.tile_pool(name="qsp", bufs=4))
                rtp = actx.enter_context(tc.tile_pool(name="rtp", bufs=2))

                sta = stap.tile([128, NP, 2, 2, SEG], f8, tag="sta", name="sta")

                def fetch_mov(c):
                    m = movp.tile([128, NP, 2, 2, CW], f8, tag="mov",
                                  name=f"mov{c}")
                    nc.sync.dma_start(m[:], mov_in[:, c])
                    return m

                # DMA order: pair 0 of mov chunk 0 + sta pair 0 first so the
                # PE can start ~3us in; the rest streams behind the first
                # pair-0 matmuls (which take ~9us at cold p-state).
                m0 = movp.tile([128, NP, 2, 2, CW], f8, tag="mov", name="mov0")
                nc.sync.dma_start(m0[:, 0], mov_in[:, 0, 0])
                nc.sync.dma_start(sta[:, 0, 0, :, 0:256], sta_in[:, 0, 0, :, 0:256])
                nc.sync.dma_start(sta[:, 0, 0, :, 256:SEG], sta_in[:, 0, 0, :, 256:SEG])
                nc.sync.dma_start(sta[:, 0, 1], sta_in[:, 0, 1])
                for p in range(1, NP):
                    nc.sync.dma_start(m0[:, p], mov_in[:, 0, p])
                    nc.sync.dma_start(sta[:, p], sta_in[:, p])
                movs = {0: m0, 1: fetch_mov(1), 2: fetch_mov(2)}
                cosb = constp.tile([128, MT, 40], bf16, tag="cosb", name="cosb")
                sinb = constp.tile([128, MT, 40], bf16, tag="sinb", name="sinb")
                ident = constp.tile([128, 128], bf16, tag="ident", name="ident")
                nc.sync.dma_start(cosb[:], cos_in[:])
                nc.sync.dma_start(sinb[:], sin_in[:])
                nc.sync.dma_start(ident[:], ident_in[:])
                for mt in range(MT):
                    nc.vector.memset(v_sb[mt][:, :, D:D + 1], 1.0)

                def emit_rope(dst, src, nh, mt):
                    # dst = src*cos + rotate_half(src)*sin, all bf16 on DVE
                    src3 = src[:, 0:nh, :]
                    src4 = src3.rearrange("p h (two d) -> p h two d", two=2)
                    cos_bc4 = (cosb[:, mt].unsqueeze(1).unsqueeze(2)
                               .broadcast_to([128, nh, 2, 40]))
                    sin_bc3 = (sinb[:, mt].unsqueeze(1)
                               .broadcast_to([128, nh, 40]))
                    t = rtp.tile([128, 6, D], bf16, tag="t", name="t")
                    t4 = t[:, 0:nh, :].rearrange("p h (two d) -> p h two d",
                                                 two=2)
                    nc.vector.tensor_mul(t4, src4, cos_bc4)
                    m1 = rtp.tile([128, 6, 40], bf16, tag="m1", name="m1")
                    nc.vector.tensor_mul(m1[:, 0:nh, :], src3[:, :, 40:80],
                                         sin_bc3)
                    m2 = rtp.tile([128, 6, 40], bf16, tag="m2", name="m2")
                    nc.vector.tensor_mul(m2[:, 0:nh, :], src3[:, :, 0:40],
                                         sin_bc3)
                    nc.vector.tensor_sub(dst[:, :, 0:40], t[:, 0:nh, 0:40],
                                         m1[:, 0:nh, :])
                    nc.vector.tensor_add(dst[:, :, 40:80], m2[:, 0:nh, :],
                                         t[:, 0:nh, 40:80])

                with tc.tile_pool(name="psA", bufs=1, space="PSUM") as psA:
                    pss = [psA.tile([128, CW], f32, tag=f"ps{mt}",
                                    name=f"ps{mt}") for mt in range(MT)]
                    terms = [(0, 0), (1, 0), (0, 1)]  # (sta lvl, mov lvl)
                    for c in range(NCH):
                        mov = movs.pop(c)
                        if c + 3 < NCH:
                            movs[c + 3] = fetch_mov(c + 3)
                        # mt-outer: each mt's full 15-matmul accumulation runs
                        # back-to-back and its eviction hides behind the next
                        # mt's matmuls, so psum reuse never stalls the PE
                        for mt in range(MT):
                            for pi in range(NP):
                                for ti, (sl, ml) in enumerate(terms):
                                    nc.tensor.matmul(
                                        pss[mt][:],
                                        sta[:, pi, sl, :,
                                            mt * 128:(mt + 1) * 128],
                                        mov[:, pi, ml, :, :],
                                        start=(pi == 0 and ti == 0),
                                        stop=(pi == NP - 1 and ti == 2),
                                        perf_mode=DR)
                            # last chunk: evict everything on DVE (fast and
                            # idle here) so psum frees fast for B's transposes
                            on_act = (mt % 2 == 0) and c != NCH - 1
                            for (kind, h0, nh, off) in _chunk_segments(c):
                                ps_sl = (pss[mt][:, off:off + nh * D]
                                         .rearrange("p (h d) -> p h d", h=nh))
                                if kind == 2:
                                    if on_act:
                                        nc.scalar.mul(
                                            v_sb[mt][:, h0:h0 + nh, 0:D],
                                            ps_sl, inv_s)
                                    else:
                                        nc.vector.tensor_scalar_mul(
                                            v_sb[mt][:, h0:h0 + nh, 0:D],
                                            ps_sl, inv_s)
                                    continue
                                qs = qsp.tile([128, 6, D], bf16, tag="qs",
                                              name="qs")
                                if on_act:
                                    nc.scalar.mul(qs[:, 0:nh, :], ps_sl, inv_s)
                                else:
                                    nc.vector.tensor_scalar_mul(
                                        qs[:, 0:nh, :], ps_sl, inv_s)
                                dst = q_sb[mt] if kind == 0 else k_sb[mt]
                                emit_rope(dst[:, h0:h0 + nh, :], qs, nh, mt)

            # ---------------- Phase B: block-diagonal attention ------------
            wpj = projp.tile([128, NP, 2, 2, HID], f8, tag="wpj", name="wpj")
            nc.sync.dma_start(wpj[:], wpj_in[:])
            # e4m3 hi/lo packed attention output, DoubleRow pair layout
            aT8 = [[projp.tile([128, 2, SEG], f8, tag=f"aT{lv}_{P}",
                               name=f"aT{lv}_{P}") for P in range(NP)]
                   for lv in range(2)]

            with ExitStack() as bctx:
                sbB = bctx.enter_context(tc.tile_pool(name="sbB", bufs=2))
                psB = bctx.enter_context(
                    tc.tile_pool(name="psB", bufs=3, space="PSUM"))
                pending = None  # (o_sb, srow) of previous head awaiting norm

                Mult = mybir.AluOpType.mult

                def emit_norm(hh, o_sb, srow):
                    # broadcast sums row to 80 partitions on gpsimd (bitcast
                    # bf16 pairs as f32 so the byte copy is dtype-agnostic),
                    # reciprocal + scaled normalize on DVE, e4m3 hi/lo split,
                    # then pack the head rows into DoubleRow-paired tiles.
                    sb80 = sbB.tile([D, SEG], bf16, tag="sb80", name="sb80")
                    nc.gpsimd.partition_broadcast(sb80[:].bitcast(f32),
                                                  srow[:].bitcast(f32))
                    rb = sbB.tile([D, SEG], bf16, tag="rb", name="rb")
                    with nc.allow_low_precision(
                            reason="softmax sums ~1e3, bf16 recip err 0.4%"):
                        nc.vector.reciprocal(rb[:], sb80[:])
                        aoT = sbB.tile([D, SEG], bf16, tag="aoT", name="aoT")
                        nc.vector.scalar_tensor_tensor(
                            aoT[:], o_sb[0:D, :], SA, rb[:], Mult, Mult)
                        hi = sbB.tile([D, SEG], f8, tag="hi", name="hi")
                        nc.vector.tensor_copy(hi[:], aoT[:])
                        lo = sbB.tile([D, SEG], f8, tag="lo", name="lo")
                        nc.vector.tensor_sub(lo[:], aoT[:], hi[:])
                    e0 = hh * D
                    pieces = []
                    while e0 < (hh + 1) * D:
                        ln = min(128 - e0 % 128, (hh + 1) * D - e0)
                        pieces.append((e0, ln))
                        e0 += ln
                    for lv, src in ((0, hi), (1, lo)):
                        for (es, ln) in pieces:
                            kc, r0 = es // 128, es % 128
                            off = es - hh * D
                            nc.sync.dma_start(
                                aT8[lv][kc // 2][r0:r0 + ln, kc % 2, :],
                                src[off:off + ln, :])

                def emit_transposes(h):
                    # bf16 psum out is exact here: transpose is a permutation
                    with nc.allow_low_precision(reason="transpose is exact"):
                        qT_ps = psB.tile([D, SEG], bf16, tag="big",
                                         name="qT_ps")
                        for mt in range(MT):
                            nc.tensor.transpose(
                                qT_ps[:, mt * 128:(mt + 1) * 128],
                                q_sb[mt][:, h, :], ident[:])
                        qT = sbB.tile([D, SEG], bf16, tag="qT", name="qT",
                                      bufs=3)
                        nc.vector.tensor_copy(qT[:], qT_ps[:])
                        kT_ps = psB.tile([D, SEG], bf16, tag="big",
                                         name="kT_ps")
                        for mt in range(MT):
                            nc.tensor.transpose(
                                kT_ps[:, mt * 128:(mt + 1) * 128],
                                k_sb[mt][:, h, :], ident[:])
                        kT = sbB.tile([D, SEG], bf16, tag="kT", name="kT")
                        nc.vector.tensor_copy(kT[:], kT_ps[:])
                    return qT, kT

                # per-head state for a 2-deep software pipeline: each head's
                # QK(0..1) are issued inside the previous head so ACT's exp
                # stream never drains at a head boundary (their s_ps slots
                # are freed by transpose copies, not by exps).
                hs = [{"p": [None] * MT, "oT": None, "T": None}
                      for _ in range(H)]

                def emit_qk(h, kc):
                    qT, kT = hs[h]["T"]
                    s_ps = psB.tile([128, SEG], f32, tag="big", name="s_ps")
                    for nn in range(2):
                        nc.tensor.matmul(
                            s_ps[:, nn * 512:(nn + 1) * 512],
                            kT[:, kc * 128:(kc + 1) * 128],
                            qT[:, nn * 512:(nn + 1) * 512],
                            start=True, stop=True)
                    p_sb = sbB.tile([128, SEG], bf16, tag="p", name="p_sb",
                                    bufs=6)
                    nc.scalar.activation(p_sb[:], s_ps[:], Exp, scale=SCALE)
                    hs[h]["p"][kc] = p_sb

                def emit_av(h, kc):
                    if hs[h]["oT"] is None:
                        hs[h]["oT"] = psB.tile([D + 1, SEG], f32, tag="oT",
                                               name="oT_ps", bufs=1)
                    for nn in range(2):
                        nc.tensor.matmul(
                            hs[h]["oT"][:, nn * 512:(nn + 1) * 512],
                            v_sb[kc][:, h, :],
                            hs[h]["p"][kc][:, nn * 512:(nn + 1) * 512],
                            start=(kc == 0), stop=(kc == MT - 1))

                def emit_fin(h):
                    o_sb = sbB.tile([D + 1, SEG], bf16, tag="o",
                                    name="o_sb", bufs=3)
                    if h == H - 1:
                        # last head: split the psum drain across DVE+ACT so
                        # phase C's psum pool unblocks sooner
                        nc.vector.tensor_copy(o_sb[:, 0:512],
                                              hs[h]["oT"][:, 0:512])
                        nc.scalar.copy(o_sb[:, 512:SEG],
                                       hs[h]["oT"][:, 512:SEG])
                    else:
                        nc.vector.tensor_copy(o_sb[:], hs[h]["oT"][:])
                    srow = sbB.tile([1, SEG], bf16, tag="srow", name="srow")
                    nc.sync.dma_start(srow[:], o_sb[D:D + 1, :])
                    return (h, o_sb, srow)

                hs[0]["T"] = emit_transposes(0)
                hs[1]["T"] = emit_transposes(1)
                emit_qk(0, 0)
                emit_qk(0, 1)
                pending = None
                for h in range(H):
                    emit_qk(h, 2)
                    emit_qk(h, 3)
                    if pending is not None:
                        emit_norm(*pending)       # norm of head h-1
                        pending = None
                    if h + 1 < H and hs[h + 1]["T"] is None:
                        hs[h + 1]["T"] = emit_transposes(h + 1)
                    emit_qk(h, 4)
                    emit_qk(h, 5)
                    emit_av(h, 0)
                    emit_qk(h, 6)
                    emit_av(h, 1)
                    emit_qk(h, 7)
                    emit_av(h, 2)
                    emit_av(h, 3)
                    emit_av(h, 4)
                    emit_av(h, 5)
                    if h + 1 < H:
                        emit_qk(h + 1, 0)
                        emit_qk(h + 1, 1)
                    emit_av(h, 6)
                    emit_av(h, 7)
                    pending = emit_fin(h)
                emit_norm(*pending)

            qkv_ctx.close()  # q/k/v dead after attention; free for phase C

            # ---------------- Phase C: output projection ----------------
            with ExitStack() as cctx:
                osbp = cctx.enter_context(tc.tile_pool(name="osbp", bufs=1))
                psC = cctx.enter_context(
                    tc.tile_pool(name="psC", bufs=1, space="PSUM"))
                NTC3 = list(enumerate([(0, 512), (512, 512), (1024, 256)]))
                # per-mt groups, psum double-buffered: each mt's outputs
                # evict + DMA out while the next mt computes. The last mt is
                # split into column halves so the final exposed evict+DMA is
                # half-sized.
                terms = [(0, 0), (1, 0), (0, 1)]  # (aT lvl, w lvl)

                bank_ctr = [0]

                def emit_proj(mts, ntc, ots, tb=0):
                    pss = {}
                    for i, mt in enumerate(mts):
                        for j, (n0, nw) in ntc:
                            pss[(i, j)] = psC.tile(
                                [128, nw], f32,
                                tag=f"b{bank_ctr[0] % 8}", name="pc", bufs=1)
                            bank_ctr[0] += 1
                    for pi in range(NP):
                        for ti, (al, wl) in enumerate(terms):
                            for i, mt in enumerate(mts):
                                for j, (n0, nw) in ntc:
                                    nc.tensor.matmul(
                                        pss[(i, j)][:],
                                        aT8[al][pi][:, :,
                                                    mt * 128:(mt + 1) * 128],
                                        wpj[:, pi, wl, :, n0:n0 + nw],
                                        start=(pi == 0 and ti == 0),
                                        stop=(pi == NP - 1 and ti == 2),
                                        perf_mode=DR)
                    c0 = min(n0 for _, (n0, _) in ntc)
                    c1 = max(n0 + nw for _, (n0, nw) in ntc)
                    for i, mt in enumerate(mts):
                        for j, (n0, nw) in ntc:
                            dst = ots[i][:, n0:n0 + nw]
                            if j == 1:
                                nc.scalar.mul(dst, pss[(i, j)][:], inv_p)
                            else:
                                nc.vector.tensor_scalar_mul(
                                    dst, pss[(i, j)][:], inv_p)
                        nc.sync.dma_start(
                            out_dram[mt * 128:(mt + 1) * 128, c0:c1],
                            ots[i][:, c0:c1])

                # first group spans 3 mts pair-outer so its pair-4 matmuls
                # land ~10us in, covering the last head's norm latency; the
                # last mt is split per column chunk to shrink the final
                # exposed evict+DMA
                ots = [osbp.tile([128, HID], f32, tag=f"osb{i}",
                                 name=f"osb{i}", bufs=1) for i in range(2)]
                emit_proj([0, 1], NTC3, ots)
                for mt in range(2, MT - 1):
                    ot = osbp.tile([128, HID], f32, tag=f"osb{mt % 2}",
                                   name="osb", bufs=1)
                    emit_proj([mt], NTC3, [ot])
                ot = osbp.tile([128, HID], f32, tag="osb2", name="osb2",
                               bufs=1)
                for j in range(3):
                    emit_proj([MT - 1], NTC3[j:j + 1], [ot])

    nc.compile()
    return nc


def _pow2scale(x):
    m = float(np.abs(x).max())
    return float(2.0 ** np.floor(np.log2(256.0 / m))) if m > 0 else 1.0


def _hilo8(x):
    import ml_dtypes
    hi = x.astype(ml_dtypes.float8_e4m3fn)
    lo = (x - hi.astype(np.float32)).astype(ml_dtypes.float8_e4m3fn)
    return hi, lo


def kernel(hidden_states, cos, sin, qkv_kernel, qkv_bias, proj_kernel,
           proj_bias, cu_seqlens):
    import ml_dtypes
    from concourse import bass_utils

    hidden_states = np.ascontiguousarray(hidden_states, dtype=np.float32)
    wqkv = np.ascontiguousarray(
        np.asarray(qkv_kernel, dtype=np.float32).reshape(HID, 3 * H * D))
    wproj = np.ascontiguousarray(proj_kernel, dtype=np.float32)

    assert not np.any(np.asarray(qkv_bias)), "nonzero qkv_bias unsupported"
    assert not np.any(np.asarray(proj_bias)), "nonzero proj_bias unsupported"
    expected_cu = np.arange(NSEG + 1, dtype=np.int64) * SEG
    assert np.array_equal(np.asarray(cu_seqlens, dtype=np.int64), expected_cu), \
        "kernel specialized for equal 1024-token segments"

    sh = _pow2scale(hidden_states)
    sw = _pow2scale(wqkv)
    swp = _pow2scale(wproj)
    inv_s = 1.0 / (sh * sw)
    inv_p = 1.0 / (SA * swp)

    key = ("nc", NSEG, inv_s, inv_p)
    if key not in _CACHE:
        _CACHE[key] = build_module(num_devices=NSEG, inv_s=inv_s, inv_p=inv_p)
    nc = _CACHE[key]

    # weights: packed per (chunk, pair, hi/lo, slot) fp8 layout
    wh, wl = _hilo8(wqkv * sw)
    mov8 = np.stack(
        [w.reshape(NP, 2, 128, NCH, CW).transpose(2, 3, 0, 1, 4)
         for w in (wh, wl)], axis=3)           # [128, NCH, NP, 2lvl, 2slot, CW]
    mov8 = np.ascontiguousarray(mov8)
    ph, pl = _hilo8(wproj * swp)
    wpj8 = np.ascontiguousarray(np.stack(
        [w.reshape(NP, 2, 128, HID).transpose(2, 0, 1, 3)
         for w in (ph, pl)], axis=2))          # [128, NP, 2lvl, 2slot, HID]
    identb = np.eye(128, dtype=ml_dtypes.bfloat16)

    in_maps = []
    for c in range(NSEG):
        rows = slice(c * SEG, (c + 1) * SEG)
        hidT = np.ascontiguousarray(hidden_states[rows].T) * sh  # [1280,1024]
        hh, hl = _hilo8(hidT)
        sta8 = np.stack(
            [x.reshape(NP, 2, 128, SEG).transpose(2, 0, 1, 3)
             for x in (hh, hl)], axis=2)       # [128, NP, 2lvl, 2slot, SEG]
        sta8 = np.ascontiguousarray(sta8)
        cosb = np.ascontiguousarray(
            np.asarray(cos[rows, 0:40]).reshape(MT, 128, 40).transpose(1, 0, 2)
            .astype(ml_dtypes.bfloat16))
        sinb = np.ascontiguousarray(
            np.asarray(sin[rows, 0:40]).reshape(MT, 128, 40).transpose(1, 0, 2)
            .astype(ml_dtypes.bfloat16))
        in_maps.append({
            "sta8": sta8,
            "mov8": mov8,
            "cosb": cosb,
            "sinb": sinb,
            "wpj8": wpj8,
            "identb": identb,
        })

    res = bass_utils.run_bass_kernel_spmd(nc, in_maps,
                                          core_ids=list(range(NSEG)))
    out = np.concatenate([res.results[c]["out"] for c in range(NSEG)], axis=0)
    return out.astype(np.float32)


# revision 59
# speedup vs baseline: 1.0022x; 1.0022x over previous
"""Trainium2 Bass kernel for ExactVisionAttention (block-diagonal attention).

Full inputs in, full outputs out. Sharding: segment-parallel over the 8
equal-length segments (attention is block-diagonal across segments), one
segment per NeuronCore. No collectives needed.

v2 dataflow per core (segment of 1024 tokens, HID=1280, 16 heads, D=80):
  Host prep (free): hidden pre-transposed to hidT and split into fp8-e4m3
  hi/lo pairs (power-of-2 prescaled); wqkv likewise; cos/sin/wproj in bf16.
  A: QKV via fp8 DoubleRow matmuls, 3 "clean" terms per 256-deep
     contraction pair: hi*hi + lo*hi + hi*lo  (~bf16 accuracy, 0.75x the
     row-cycles of bf16). PSUM evict on ACT with the descale folded in;
     RoPE on DVE in bf16 (4x mode); V eviction straight to bf16 (+ones col).
  B: per head on PE: transpose q,k to [80,1024] (bf16, 1cyc/row); S^T =
     kT.T@qT chunks; exp on ACT (scale folded, no max-subtraction);
     oT = [V|1].T@P^T accumulated; row 80 = softmax sums. Normalize via
     gpsimd partition-broadcast + DVE reciprocal/mul (bf16); pack the
     normalized aoT head rows into 128-partition e-major SBUF tiles with
     SBUF->SBUF DMA (no DRAM scratch round trip).
  C: output projection, also fp8 DoubleRow 3-term: the normalized head
     outputs are split hi/lo on DVE in phase B and packed into DoubleRow
     pair layout; wproj hi/lo comes prescaled from the host (prefetched
     during B). A leading 2-mt pair-outer group covers the last head's
     norm latency; descale folds into the f32 eviction before DMA out.

qkv_bias / proj_bias are zeros by construction (spec fill=zeros) and are
not applied. cu_seqlens is fixed equal segmentation and only validated.
"""

import os
import sys

for _p in ("/opt/trn_rl_repo", "/root/.axon_site", "/root/.axon_site/_ro/trn_rl_repo",
           "/root/.axon_site/_ro/pypackages"):
    if os.path.isdir(_p) and _p not in sys.path:
        sys.path.append(_p)

import numpy as np

S = 8192
HID = 1280
H = 16
D = 80
NSEG = 8
SEG = S // NSEG          # 1024 tokens per segment/core
MT = SEG // 128          # 8 token tiles per core
KC = HID // 128          # 10 proj contraction chunks
NP = 5                   # qkv contraction pairs (256 deep each)
CW = 480                 # qkv output column chunk (6 heads)
NCH = 3 * HID // CW      # 8 column chunks
SCALE = float(D) ** -0.5

_CACHE = {}


def _chunk_segments(c):
    """Column chunk c -> [(kind, head0, nheads, col_off_in_chunk)]."""
    segs = []
    col = c * CW
    end = col + CW
    while col < end:
        kind = col // HID
        base = kind * HID
        h0 = (col - base) // D
        ncols = min(end, base + HID) - col
        nh = ncols // D
        segs.append((kind, h0, nh, col - c * CW))
        col += nh * D
    return segs


SA = 64.0  # fixed pow2 prescale for the normalized attention output


def build_module(num_devices=8, repeat=1, inv_s=1.0 / (32.0 * 2048.0),
                 inv_p=1.0 / (64.0 * 2048.0)):
    import concourse.tile as tile
    from concourse import bacc, mybir

    f32 = mybir.dt.float32
    bf16 = mybir.dt.bfloat16
    f8 = mybir.dt.float8e4
    Exp = mybir.ActivationFunctionType.Exp
    DR = mybir.MatmulPerfMode.DoubleRow

    nc = bacc.Bacc("TRN2", target_bir_lowering=False, debug=False,
                   num_devices=num_devices)

    sta_in = nc.dram_tensor("sta8", [128, NP, 2, 2, SEG], f8,
                            kind="ExternalInput").ap()
    mov_in = nc.dram_tensor("mov8", [128, NCH, NP, 2, 2, CW], f8,
                            kind="ExternalInput").ap()
    cos_in = nc.dram_tensor("cosb", [128, MT, 40], bf16,
                            kind="ExternalInput").ap()
    sin_in = nc.dram_tensor("sinb", [128, MT, 40], bf16,
                            kind="ExternalInput").ap()
    wpj_in = nc.dram_tensor("wpj8", [128, NP, 2, 2, HID], f8,
                            kind="ExternalInput").ap()
    ident_in = nc.dram_tensor("identb", [128, 128], bf16,
                              kind="ExternalInput").ap()
    out_dram = nc.dram_tensor("out", [SEG, HID], f32, kind="ExternalOutput").ap()
    tag_dram = None
    if repeat > 1:
        tag_dram = nc.dram_tensor("rtag", [1, repeat], f32,
                                  kind="ExternalOutput").ap()

    with tile.TileContext(nc) as tc:
      from contextlib import ExitStack
      for _rep in range(repeat):
        with ExitStack() as ctx:
            constp = ctx.enter_context(tc.tile_pool(name="const", bufs=1))
            projp = ctx.enter_context(tc.tile_pool(name="projp", bufs=1))
            qkv_ctx = ExitStack()
            qkvsb = qkv_ctx.enter_context(tc.tile_pool(name="qkvsb", bufs=1))

            from concourse import library_config
            nc.gpsimd.load_library(library_config.proxy)

            q_sb = [qkvsb.tile([128, H, D], bf16, tag=f"q{mt}", name=f"q{mt}")
                    for mt in range(MT)]
            k_sb = [qkvsb.tile([128, H, D], bf16, tag=f"k{mt}", name=f"k{mt}")
                    for mt in range(MT)]
            v_sb = [qkvsb.tile([128, H, D + 1], bf16, tag=f"v{mt}", name=f"v{mt}")
                    for mt in range(MT)]

            if tag_dram is not None:
                nc.sync.dma_start(tag_dram[:, _rep:_rep + 1],
                                  cos_in[0:1, 0:1, 0:1].rearrange("a b c -> a (b c)"))

            # ---------------- Phase A: QKV (fp8 DoubleRow) + RoPE ----------
            with ExitStack() as actx:
                stap = actx.enter_context(tc.tile_pool(name="stap", bufs=1))
                movp = actx.enter_context(tc.tile_pool(name="movp", bufs=3))
                qsp = actx.enter_context(tc.tile_pool(name="qsp", bufs=4))
                rtp = actx.enter_context(tc.tile_pool(name="rtp", bufs=2))

                sta = stap.tile([128, NP, 2, 2, SEG], f8, tag="sta", name="sta")

                def fetch_mov(c):
                    m = movp.tile([128, NP, 2, 2, CW], f8, tag="mov",
                                  name=f"mov{c}")
                    nc.sync.dma_start(m[:], mov_in[:, c])
                    return m

                # DMA order: pair 0 of mov chunk 0 + sta pair 0 first so the
                # PE can start ~3us in; the rest streams behind the first
                # pair-0 matmuls (which take ~9us at cold p-state).
                m0 = movp.tile([128, NP, 2, 2, CW], f8, tag="mov", name="mov0")
                nc.sync.dma_start(m0[:, 0], mov_in[:, 0, 0])
                nc.sync.dma_start(sta[:, 0, 0], sta_in[:, 0, 0])
                nc.sync.dma_start(sta[:, 0, 1], sta_in[:, 0, 1])
                for p in range(1, NP):
                    nc.sync.dma_start(m0[:, p], mov_in[:, 0, p])
                    nc.sync.dma_start(sta[:, p], sta_in[:, p])
                movs = {0: m0, 1: fetch_mov(1), 2: fetch_mov(2)}
                cosb = constp.tile([128, MT, 40], bf16, tag="cosb", name="cosb")
                sinb = constp.tile([128, MT, 40], bf16, tag="sinb", name="sinb")
                ident = constp.tile([128, 128], bf16, tag="ident", name="ident")
                nc.sync.dma_start(cosb[:], cos_in[:])
                nc.sync.dma_start(sinb[:], sin_in[:])
                nc.sync.dma_start(ident[:], ident_in[:])
                for mt in range(MT):
                    nc.vector.memset(v_sb[mt][:, :, D:D + 1], 1.0)

                def emit_rope(dst, src, nh, mt):
                    # dst = src*cos + rotate_half(src)*sin, all bf16 on DVE
                    src3 = src[:, 0:nh, :]
                    src4 = src3.rearrange("p h (two d) -> p h two d", two=2)
                    cos_bc4 = (cosb[:, mt].unsqueeze(1).unsqueeze(2)
                               .broadcast_to([128, nh, 2, 40]))
                    sin_bc3 = (sinb[:, mt].unsqueeze(1)
                               .broadcast_to([128, nh, 40]))
                    t = rtp.tile([128, 6, D], bf16, tag="t", name="t")
                    t4 = t[:, 0:nh, :].rearrange("p h (two d) -> p h two d",
                                                 two=2)
                    nc.vector.tensor_mul(t4, src4, cos_bc4)
                    m1 = rtp.tile([128, 6, 40], bf16, tag="m1", name="m1")
                    nc.vector.tensor_mul(m1[:, 0:nh, :], src3[:, :, 40:80],
                                         sin_bc3)
                    m2 = rtp.tile([128, 6, 40], bf16, tag="m2", name="m2")
                    nc.vector.tensor_mul(m2[:, 0:nh, :], src3[:, :, 0:40],
                                         sin_bc3)
                    nc.vector.tensor_sub(dst[:, :, 0:40], t[:, 0:nh, 0:40],
                                         m1[:, 0:nh, :])
                    nc.vector.tensor_add(dst[:, :, 40:80], m2[:, 0:nh, :],
                                         t[:, 0:nh, 40:80])

                with tc.tile_pool(name="psA", bufs=1, space="PSUM") as psA:
                    pss = [psA.tile([128, CW], f32, tag=f"ps{mt}",
                                    name=f"ps{mt}") for mt in range(MT)]
                    terms = [(0, 0), (1, 0), (0, 1)]  # (sta lvl, mov lvl)
                    for c in range(NCH):
                        mov = movs.pop(c)
                        if c + 3 < NCH:
                            movs[c + 3] = fetch_mov(c + 3)
                        # mt-outer: each mt's full 15-matmul accumulation runs
                        # back-to-back and its eviction hides behind the next
                        # mt's matmuls, so psum reuse never stalls the PE
                        for mt in range(MT):
                            for pi in range(NP):
                                for ti, (sl, ml) in enumerate(terms):
                                    nc.tensor.matmul(
                                        pss[mt][:],
                                        sta[:, pi, sl, :,
                                            mt * 128:(mt + 1) * 128],
                                        mov[:, pi, ml, :, :],
                                        start=(pi == 0 and ti == 0),
                                        stop=(pi == NP - 1 and ti == 2),
                                        perf_mode=DR)
                            # last chunk: evict everything on DVE (fast and
                            # idle here) so psum frees fast for B's transposes
                            on_act = (mt % 2 == 0) and c != NCH - 1
                            for (kind, h0, nh, off) in _chunk_segments(c):
                                ps_sl = (pss[mt][:, off:off + nh * D]
                                         .rearrange("p (h d) -> p h d", h=nh))
                                if kind == 2:
                                    if on_act:
                                        nc.scalar.mul(
                                            v_sb[mt][:, h0:h0 + nh, 0:D],
                                            ps_sl, inv_s)
                                    else:
                                        nc.vector.tensor_scalar_mul(
                                            v_sb[mt][:, h0:h0 + nh, 0:D],
                                            ps_sl, inv_s)
                                    continue
                                qs = qsp.tile([128, 6, D], bf16, tag="qs",
                                              name="qs")
                                if on_act:
                                    nc.scalar.mul(qs[:, 0:nh, :], ps_sl, inv_s)
                                else:
                                    nc.vector.tensor_scalar_mul(
                                        qs[:, 0:nh, :], ps_sl, inv_s)
                                dst = q_sb[mt] if kind == 0 else k_sb[mt]
                                emit_rope(dst[:, h0:h0 + nh, :], qs, nh, mt)

            # ---------------- Phase B: block-diagonal attention ------------
            wpj = projp.tile([128, NP, 2, 2, HID], f8, tag="wpj", name="wpj")
            nc.sync.dma_start(wpj[:], wpj_in[:])
            # e4m3 hi/lo packed attention output, DoubleRow pair layout
            aT8 = [[projp.tile([128, 2, SEG], f8, tag=f"aT{lv}_{P}",
                               name=f"aT{lv}_{P}") for P in range(NP)]
                   for lv in range(2)]

            with ExitStack() as bctx:
                sbB = bctx.enter_context(tc.tile_pool(name="sbB", bufs=2))
                psB = bctx.enter_context(
                    tc.tile_pool(name="psB", bufs=3, space="PSUM"))
                pending = None  # (o_sb, srow) of previous head awaiting norm

                Mult = mybir.AluOpType.mult

                def emit_norm(hh, o_sb, srow):
                    # broadcast sums row to 80 partitions on gpsimd (bitcast
                    # bf16 pairs as f32 so the byte copy is dtype-agnostic),
                    # reciprocal + scaled normalize on DVE, e4m3 hi/lo split,
                    # then pack the head rows into DoubleRow-paired tiles.
                    sb80 = sbB.tile([D, SEG], bf16, tag="sb80", name="sb80")
                    nc.gpsimd.partition_broadcast(sb80[:].bitcast(f32),
                                                  srow[:].bitcast(f32))
                    rb = sbB.tile([D, SEG], bf16, tag="rb", name="rb")
                    with nc.allow_low_precision(
                            reason="softmax sums ~1e3, bf16 recip err 0.4%"):
                        nc.vector.reciprocal(rb[:], sb80[:])
                        aoT = sbB.tile([D, SEG], bf16, tag="aoT", name="aoT")
                        nc.vector.scalar_tensor_tensor(
                            aoT[:], o_sb[0:D, :], SA, rb[:], Mult, Mult)
                        hi = sbB.tile([D, SEG], f8, tag="hi", name="hi")
                        nc.vector.tensor_copy(hi[:], aoT[:])
                        lo = sbB.tile([D, SEG], f8, tag="lo", name="lo")
                        nc.vector.tensor_sub(lo[:], aoT[:], hi[:])
                    e0 = hh * D
                    pieces = []
                    while e0 < (hh + 1) * D:
                        ln = min(128 - e0 % 128, (hh + 1) * D - e0)
                        pieces.append((e0, ln))
                        e0 += ln
                    for lv, src in ((0, hi), (1, lo)):
                        for (es, ln) in pieces:
                            kc, r0 = es // 128, es % 128
                            off = es - hh * D
                            nc.sync.dma_start(
                                aT8[lv][kc // 2][r0:r0 + ln, kc % 2, :],
                                src[off:off + ln, :])

                def emit_transposes(h):
                    # bf16 psum out is exact here: transpose is a permutation
                    with nc.allow_low_precision(reason="transpose is exact"):
                        qT_ps = psB.tile([D, SEG], bf16, tag="big",
                                         name="qT_ps")
                        for mt in range(MT):
                            nc.tensor.transpose(
                                qT_ps[:, mt * 128:(mt + 1) * 128],
                                q_sb[mt][:, h, :], ident[:])
                        qT = sbB.tile([D, SEG], bf16, tag="qT", name="qT",
                                      bufs=3)
                        nc.vector.tensor_copy(qT[:], qT_ps[:])
                        kT_ps = psB.tile([D, SEG], bf16, tag="big",
                                         name="kT_ps")
                        for mt in range(MT):
                            nc.tensor.transpose(
                                kT_ps[:, mt * 128:(mt + 1) * 128],
                                k_sb[mt][:, h, :], ident[:])
                        kT = sbB.tile([D, SEG], bf16, tag="kT", name="kT")
                        nc.vector.tensor_copy(kT[:], kT_ps[:])
                    return qT, kT

                # per-head state for a 2-deep software pipeline: each head's
                # QK(0..1) are issued inside the previous head so ACT's exp
                # stream never drains at a head boundary (their s_ps slots
                # are freed by transpose copies, not by exps).
                hs = [{"p": [None] * MT, "oT": None, "T": None}
                      for _ in range(H)]

                def emit_qk(h, kc):
                    qT, kT = hs[h]["T"]
                    s_ps = psB.tile([128, SEG], f32, tag="big", name="s_ps")
                    for nn in range(2):
                        nc.tensor.matmul(
                            s_ps[:, nn * 512:(nn + 1) * 512],
                            kT[:, kc * 128:(kc + 1) * 128],
                            qT[:, nn * 512:(nn + 1) * 512],
                            start=True, stop=True)
                    p_sb = sbB.tile([128, SEG], bf16, tag="p", name="p_sb",
                                    bufs=6)
                    nc.scalar.activation(p_sb[:], s_ps[:], Exp, scale=SCALE)
                    hs[h]["p"][kc] = p_sb

                def emit_av(h, kc):
                    if hs[h]["oT"] is None:
                        hs[h]["oT"] = psB.tile([D + 1, SEG], f32, tag="oT",
                                               name="oT_ps", bufs=1)
                    for nn in range(2):
                        nc.tensor.matmul(
                            hs[h]["oT"][:, nn * 512:(nn + 1) * 512],
                            v_sb[kc][:, h, :],
                            hs[h]["p"][kc][:, nn * 512:(nn + 1) * 512],
                            start=(kc == 0), stop=(kc == MT - 1))

                def emit_fin(h):
                    o_sb = sbB.tile([D + 1, SEG], bf16, tag="o",
                                    name="o_sb", bufs=3)
                    if h == H - 1:
                        # last head: split the psum drain across DVE+ACT so
                        # phase C's psum pool unblocks sooner
                        nc.vector.tensor_copy(o_sb[:, 0:512],
                                              hs[h]["oT"][:, 0:512])
                        nc.scalar.copy(o_sb[:, 512:SEG],
                                       hs[h]["oT"][:, 512:SEG])
                    else:
                        nc.vector.tensor_copy(o_sb[:], hs[h]["oT"][:])
                    srow = sbB.tile([1, SEG], bf16, tag="srow", name="srow")
                    nc.sync.dma_start(srow[:], o_sb[D:D + 1, :])
                    return (h, o_sb, srow)

                hs[0]["T"] = emit_transposes(0)
                hs[1]["T"] = emit_transposes(1)
                emit_qk(0, 0)
                emit_qk(0, 1)
                pending = None
                for h in range(H):
                    emit_qk(h, 2)
                    emit_qk(h, 3)
                    if pending is not None:
                        emit_norm(*pending)       # norm of head h-1
                        pending = None
                    if h + 1 < H and hs[h + 1]["T"] is None:
                        hs[h + 1]["T"] = emit_transposes(h + 1)
                    emit_qk(h, 4)
                    emit_qk(h, 5)
                    emit_av(h, 0)
                    emit_qk(h, 6)
                    emit_av(h, 1)
                    emit_qk(h, 7)
                    emit_av(h, 2)
                    emit_av(h, 3)
                    emit_av(h, 4)
                    emit_av(h, 5)
                    if h + 1 < H:
                        emit_qk(h + 1, 0)
                        emit_qk(h + 1, 1)
                    emit_av(h, 6)
                    emit_av(h, 7)
                    pending = emit_fin(h)
                emit_norm(*pending)

            qkv_ctx.close()  # q/k/v dead after attention; free for phase C

            # ---------------- Phase C: output projection ----------------
            with ExitStack() as cctx:
                osbp = cctx.enter_context(tc.tile_pool(name="osbp", bufs=1))
                psC = cctx.enter_context(
                    tc.tile_pool(name="psC", bufs=1, space="PSUM"))
                NTC3 = list(enumerate([(0, 512), (512, 512), (1024, 256)]))
                # per-mt groups, psum double-buffered: each mt's outputs
                # evict + DMA out while the next mt computes. The last mt is
                # split into column halves so the final exposed evict+DMA is
                # half-sized.
                terms = [(0, 0), (1, 0), (0, 1)]  # (aT lvl, w lvl)

                bank_ctr = [0]

                def emit_proj(mts, ntc, ots, tb=0):
                    pss = {}
                    for i, mt in enumerate(mts):
                        for j, (n0, nw) in ntc:
                            pss[(i, j)] = psC.tile(
                                [128, nw], f32,
                                tag=f"b{bank_ctr[0] % 8}", name="pc", bufs=1)
                            bank_ctr[0] += 1
                    for pi in range(NP):
                        for ti, (al, wl) in enumerate(terms):
                            for i, mt in enumerate(mts):
                                for j, (n0, nw) in ntc:
                                    nc.tensor.matmul(
                                        pss[(i, j)][:],
                                        aT8[al][pi][:, :,
                                                    mt * 128:(mt + 1) * 128],
                                        wpj[:, pi, wl, :, n0:n0 + nw],
                                        start=(pi == 0 and ti == 0),
                                        stop=(pi == NP - 1 and ti == 2),
                                        perf_mode=DR)
                    c0 = min(n0 for _, (n0, _) in ntc)
                    c1 = max(n0 + nw for _, (n0, nw) in ntc)
                    for i, mt in enumerate(mts):
                        for j, (n0, nw) in ntc:
                            dst = ots[i][:, n0:n0 + nw]
                            if j == 1:
                                nc.scalar.mul(dst, pss[(i, j)][:], inv_p)
                            else:
                                nc.vector.tensor_scalar_mul(
                                    dst, pss[(i, j)][:], inv_p)
                        nc.sync.dma_start(
                            out_dram[mt * 128:(mt + 1) * 128, c0:c1],
                            ots[i][:, c0:c1])

                # first group spans 3 mts pair-outer so its pair-4 matmuls
                # land ~10us in, covering the last head's norm latency; the
                # last mt is split per column chunk to shrink the final
                # exposed evict+DMA
                ots = [osbp.tile([128, HID], f32, tag=f"osb{i}",
                                 name=f"osb{i}", bufs=1) for i in range(2)]
                emit_proj([0, 1], NTC3, ots)
                for mt in range(2, MT - 1):
                    ot = osbp.tile([128, HID], f32, tag=f"osb{mt % 2}",
                                   name="osb", bufs=1)
                    emit_proj([mt], NTC3, [ot])
                ot = osbp.tile([128, HID], f32, tag="osb2", name="osb2",
                               bufs=1)
                for j in range(3):
                    emit_proj([MT - 1], NTC3[j:j + 1], [ot])

    nc.compile()
    return nc


def _pow2scale(x):
    m = float(np.abs(x).max())
    return float(2.0 ** np.floor(np.log2(256.0 / m))) if m > 0 else 1.0


def _hilo8(x):
    import ml_dtypes
    hi = x.astype(ml_dtypes.float8_e4m3fn)
    lo = (x - hi.astype(np.float32)).astype(ml_dtypes.float8_e4m3fn)
    return hi, lo


def kernel(hidden_states, cos, sin, qkv_kernel, qkv_bias, proj_kernel,
           proj_bias, cu_seqlens):
    import ml_dtypes
    from concourse import bass_utils

    hidden_states = np.ascontiguousarray(hidden_states, dtype=np.float32)
    wqkv = np.ascontiguousarray(
        np.asarray(qkv_kernel, dtype=np.float32).reshape(HID, 3 * H * D))
    wproj = np.ascontiguousarray(proj_kernel, dtype=np.float32)

    assert not np.any(np.asarray(qkv_bias)), "nonzero qkv_bias unsupported"
    assert not np.any(np.asarray(proj_bias)), "nonzero proj_bias unsupported"
    expected_cu = np.arange(NSEG + 1, dtype=np.int64) * SEG
    assert np.array_equal(np.asarray(cu_seqlens, dtype=np.int64), expected_cu), \
        "kernel specialized for equal 1024-token segments"

    sh = _pow2scale(hidden_states)
    sw = _pow2scale(wqkv)
    swp = _pow2scale(wproj)
    inv_s = 1.0 / (sh * sw)
    inv_p = 1.0 / (SA * swp)

    key = ("nc", NSEG, inv_s, inv_p)
    if key not in _CACHE:
        _CACHE[key] = build_module(num_devices=NSEG, inv_s=inv_s, inv_p=inv_p)
    nc = _CACHE[key]

    # weights: packed per (chunk, pair, hi/lo, slot) fp8 layout
    wh, wl = _hilo8(wqkv * sw)
    mov8 = np.stack(
        [w.reshape(NP, 2, 128, NCH, CW).transpose(2, 3, 0, 1, 4)
         for w in (wh, wl)], axis=3)           # [128, NCH, NP, 2lvl, 2slot, CW]
    mov8 = np.ascontiguousarray(mov8)
    ph, pl = _hilo8(wproj * swp)
    wpj8 = np.ascontiguousarray(np.stack(
        [w.reshape(NP, 2, 128, HID).transpose(2, 0, 1, 3)
         for w in (ph, pl)], axis=2))          # [128, NP, 2lvl, 2slot, HID]
    identb = np.eye(128, dtype=ml_dtypes.bfloat16)

    in_maps = []
    for c in range(NSEG):
        rows = slice(c * SEG, (c + 1) * SEG)
        hidT = np.ascontiguousarray(hidden_states[rows].T) * sh  # [1280,1024]
        hh, hl = _hilo8(hidT)
        sta8 = np.stack(
            [x.reshape(NP, 2, 128, SEG).transpose(2, 0, 1, 3)
             for x in (hh, hl)], axis=2)       # [128, NP, 2lvl, 2slot, SEG]
        sta8 = np.ascontiguousarray(sta8)
        cosb = np.ascontiguousarray(
            np.asarray(cos[rows, 0:40]).reshape(MT, 128, 40).transpose(1, 0, 2)
            .astype(ml_dtypes.bfloat16))
        sinb = np.ascontiguousarray(
            np.asarray(sin[rows, 0:40]).reshape(MT, 128, 40).transpose(1, 0, 2)
            .astype(ml_dtypes.bfloat16))
        in_maps.append({
            "sta8": sta8,
            "mov8": mov8,
            "cosb": cosb,
            "sinb": sinb,
            "wpj8": wpj8,
            "identb": identb,
        })

    res = bass_utils.run_bass_kernel_spmd(nc, in_maps,
                                          core_ids=list(range(NSEG)))
    out = np.concatenate([res.results[c]["out"] for c in range(NSEG)], axis=0)
    return out.astype(np.float32)
